# revision 8
# baseline (speedup 1.0000x reference)
"""Fused QKV projection + correlation attention (softmax over keys) on 8 trn2 cores.

Problem: x[4,2048,1024] f32; K/Q/V = x@W* + b*; out = softmax(Q Kt / 32, keys) @ V.

Sharding: core c -> batch b=c//2, key-half h=c%2.  Each core:
  - projects K,V for its 1024-key half, Q for all 2048 queries of its batch
  - computes U = exp(Q Kt/32) @ V  (unnormalized) and rs = rowsum(exp(..))
Host combines per-batch:  out[b] = (U0+U1)/(rs0+rs1)[:,None] + bv
(no max-subtraction needed: scores ~ N(0,1), exp stays within fp32 range).

Device layouts (partition dim first):
  xkvT [d, sk]   xqT [d, sq]  (host pre-transposed; projections contract over
  d on the partition axis, so x must appear transposed -- free on the host)
  KT[dout, sk], QT[dout, sq] from lhsT=W chunk;  V[sk, d] from lhsT=xkvT chunk
  scoresT[sk, sq] from lhsT=KT chunk, rhs=QT -> exp on ACT (scale=1/32 folded)
  U[sq, d] from lhsT=expT chunk, rhs=V;  rs via lhsT=ones[128,1], rhs=expT.

Matmul dtype `MM`: float32r (full PE rate at N=512, needs producers to round
-> DMA-loaded x/W pass through a DVE/ACT copy) or bfloat16 (host casts x/W).
"""

import numpy as np

B, S, D = 4, 2048, 1024
N_CORES = 8
MM = "float32r"  # "float32r" | "bfloat16" | "float32"

_BUILD_CACHE = {}
_RUN_KWARGS = {}      # test.py sets {"trace": True, ...} for profiling runs
_LAST_RESULTS = None  # BassKernelResults of the last run


def _build(d, sk, sq, mm=MM):
    """Build the per-core module. d: model dim; sk: keys/core; sq: queries/core."""
    key = (d, sk, sq, mm)
    if key in _BUILD_CACHE:
        return _BUILD_CACHE[key]

    from contextlib import ExitStack

    import concourse.bass as bass  # noqa: F401
    import concourse.mybir as mybir
    from concourse import bacc
    from concourse.tile import TileContext

    f32 = mybir.dt.float32
    mmdt = getattr(mybir.dt, mm)
    rounded = mm == "float32r"   # DMA-loaded operands need a rounding copy
    in_dt = mmdt if mm == "bfloat16" else f32  # dram dtype of x / W inputs

    P = 128
    NFREE = 512  # max fp32 moving free dim / one psum bank
    DC = d // P              # d chunks (contraction + dout chunks)
    KC = sk // P             # key chunks
    BLK = min(sq, NFREE)     # sq block width
    NBLK = sq // BLK
    SQ4 = BLK // P           # 128-row sq chunks per block
    NKB = max(1, sk // NFREE)
    KB = min(sk, NFREE)
    ND = max(1, d // NFREE)
    DB = min(d, NFREE)
    scale = float(1.0 / np.sqrt(np.float32(d)))

    nc = bacc.Bacc("TRN2", target_bir_lowering=False)
    Ident = mybir.ActivationFunctionType.Identity
    Exp = mybir.ActivationFunctionType.Exp

    xkvT = nc.dram_tensor("xkvT", [d, sk], in_dt, kind="ExternalInput")
    xqT = nc.dram_tensor("xqT", [d, sq], in_dt, kind="ExternalInput")
    Wk = nc.dram_tensor("Wk", [d, d], in_dt, kind="ExternalInput")
    Wq = nc.dram_tensor("Wq", [d, d], in_dt, kind="ExternalInput")
    Wv = nc.dram_tensor("Wv", [d, d], in_dt, kind="ExternalInput")
    bk = nc.dram_tensor("bk", [d], f32, kind="ExternalInput")
    bq = nc.dram_tensor("bq", [d], f32, kind="ExternalInput")
    U = nc.dram_tensor("U", [sq, d], f32, kind="ExternalOutput")
    rs = nc.dram_tensor("rs", [sq], f32, kind="ExternalOutput")

    xkvT_v = xkvT.ap().rearrange("(c p) s -> c p s", p=P)
    xqT_v = xqT.ap().rearrange("(c p) s -> c p s", p=P)
    Wk_v = Wk.ap().rearrange("(c p) e -> c p e", p=P)
    Wq_v = Wq.ap().rearrange("(c p) e -> p c e", p=P)  # [128, DC, d]
    Wv_v = Wv.ap().rearrange("(c p) e -> c p e", p=P)

    with TileContext(nc) as tc, ExitStack() as outer:
        resid = outer.enter_context(tc.tile_pool(name="resid", bufs=1))

        KT_sb = resid.tile([P, DC, sk], mmdt)     # [dout, sk]
        V_sb = resid.tile([P, KC, d], mmdt)       # [sk, d]
        bk_sb = resid.tile([P, DC], f32)
        bq_sb = resid.tile([P, DC], f32)
        ones_f = resid.tile([P, 1], f32)
        ones_sb = resid.tile([P, 1], mmdt)
        rs_stage = resid.tile([1, sq], f32)

        nc.vector.memset(ones_f, 1.0)
        nc.vector.tensor_copy(ones_sb, ones_f)
        nc.sync.dma_start(out=bk_sb, in_=bk.ap().rearrange("(c p) -> p c", p=P))
        nc.sync.dma_start(out=bq_sb, in_=bq.ap().rearrange("(c p) -> p c", p=P))

        def load(pool, stg_pool, dram_ap, shape, name, engine):
            """DMA dram -> mmdt tile, rounding through f32 staging if needed."""
            t = pool.tile([P, *shape], mmdt, name=name)
            if rounded:
                stg = stg_pool.tile([P, *shape], f32, name=f"{name}_stg")
                nc.sync.dma_start(out=stg, in_=dram_ap)
                engine(t, stg)
            else:
                nc.sync.dma_start(out=t, in_=dram_ap)
            return t

        # ---------------- stage 0: K and V projections (key half) ----------
        with ExitStack() as s0:
            p0 = s0.enter_context(tc.tile_pool(name="p0", bufs=1))
            stg0 = s0.enter_context(tc.tile_pool(name="stg0", bufs=4))
            ps0 = s0.enter_context(tc.tile_pool(name="ps0", bufs=4, space="PSUM"))

            xkv_sb = p0.tile([P, DC, sk], mmdt)
            Wk_sb = p0.tile([P, DC, d], mmdt)
            Wv_sb = p0.tile([P, DC, d], mmdt)
            for c in range(DC):
                for dst, src, eng in (
                    (xkv_sb, xkvT_v, nc.vector.tensor_copy),
                    (Wk_sb, Wk_v, nc.scalar.copy),
                    (Wv_sb, Wv_v, nc.scalar.copy),
                ):
                    if rounded:
                        stg = stg0.tile([P, max(sk, d)], f32, name="stg")
                        nc.sync.dma_start(out=stg[:, :src[c].shape[-1]], in_=src[c])
                        eng(dst[:, c, :], stg[:, :src[c].shape[-1]])
                    else:
                        nc.sync.dma_start(out=dst[:, c, :], in_=src[c])

            # KT[dout m, sk] = sum_k Wk[k,m]^T xkv[k,:]   (+bk on evacuation)
            for m in range(DC):
                for nb in range(NKB):
                    ps = ps0.tile([P, KB], f32, name="ps_proj")
                    for k in range(DC):
                        nc.tensor.matmul(
                            ps,
                            Wk_sb[:, k, m * P:(m + 1) * P],
                            xkv_sb[:, k, nb * KB:(nb + 1) * KB],
                            start=(k == 0), stop=(k == DC - 1),
                        )
                    nc.scalar.activation(
                        KT_sb[:, m, nb * KB:(nb + 1) * KB], ps, Ident,
                        bias=bk_sb[:, m:m + 1], scale=1.0,
                    )
            # V[sk m, d] = sum_k xkv[k,m]^T Wv[k,:]   (bv added on host)
            for m in range(KC):
                for nb in range(ND):
                    ps = ps0.tile([P, DB], f32, name="ps_proj")
                    for k in range(DC):
                        nc.tensor.matmul(
                            ps,
                            xkv_sb[:, k, m * P:(m + 1) * P],
                            Wv_sb[:, k, nb * DB:(nb + 1) * DB],
                            start=(k == 0), stop=(k == DC - 1),
                        )
                    nc.vector.tensor_copy(V_sb[:, m, nb * DB:(nb + 1) * DB], ps)

        # ---------------- stage 1: per sq-block Q proj, scores, exp, AV ----
        with ExitStack() as s1:
            pwq = s1.enter_context(tc.tile_pool(name="pwq", bufs=3))
            stgq = s1.enter_context(tc.tile_pool(name="stgq", bufs=2))
            pxq = s1.enter_context(tc.tile_pool(name="pxq", bufs=2))
            pqt = s1.enter_context(tc.tile_pool(name="pqt", bufs=2))
            pexp = s1.enter_context(tc.tile_pool(name="pexp", bufs=2))
            pout = s1.enter_context(tc.tile_pool(name="pout", bufs=4))
            ps_sh = s1.enter_context(tc.tile_pool(name="ps_sh", bufs=3, space="PSUM"))
            ps_av = s1.enter_context(tc.tile_pool(name="ps_av", bufs=4, space="PSUM"))

            for blk in range(NBLK):
                lo = blk * BLK
                # Q projection inputs for this block of queries
                xq_blk = pxq.tile([P, DC, BLK], mmdt)
                for c in range(DC):
                    if rounded:
                        stg = stgq.tile([P, BLK], f32, name="stg_xq")
                        nc.sync.dma_start(out=stg, in_=xqT_v[c][:, lo:lo + BLK])
                        nc.vector.tensor_copy(xq_blk[:, c, :], stg)
                    else:
                        nc.sync.dma_start(
                            out=xq_blk[:, c, :], in_=xqT_v[c][:, lo:lo + BLK])
                qt_blk = pqt.tile([P, DC, BLK], mmdt)
                for m in range(DC):
                    wq_m = load(
                        pwq, stgq, Wq_v[:, :, m * P:(m + 1) * P],
                        [DC, P], "wq_m", nc.scalar.copy,
                    )
                    ps = ps_sh.tile([P, BLK], f32, name="ps_q", tag="ps_sh")
                    for k in range(DC):
                        nc.tensor.matmul(
                            ps, wq_m[:, k, :], xq_blk[:, k, :],
                            start=(k == 0), stop=(k == DC - 1),
                        )
                    nc.scalar.activation(
                        qt_blk[:, m, :], ps, Ident,
                        bias=bq_sb[:, m:m + 1], scale=1.0,
                    )
                # scoresT + exp:  expT[sk, sq_blk] = exp(scale * KT^T Q)
                exp_blk = pexp.tile([P, KC, BLK], mmdt)
                for skc in range(KC):
                    ps = ps_sh.tile([P, BLK], f32, name="ps_s", tag="ps_sh")
                    for dc in range(DC):
                        nc.tensor.matmul(
                            ps,
                            KT_sb[:, dc, skc * P:(skc + 1) * P],
                            qt_blk[:, dc, :],
                            start=(dc == 0), stop=(dc == DC - 1),
                        )
                    nc.scalar.activation(
                        exp_blk[:, skc, :], ps, Exp, bias=0.0, scale=scale,
                    )
                # row sums: rs[sq_blk] = sum_sk exp  (ones is a 1-col lhsT)
                ps_rs = ps_sh.tile([1, BLK], f32, name="ps_rs", tag="ps_sh")
                for skc in range(KC):
                    nc.tensor.matmul(
                        ps_rs, ones_sb, exp_blk[:, skc, :],
                        start=(skc == 0), stop=(skc == KC - 1),
                    )
                nc.vector.tensor_copy(rs_stage[:, lo:lo + BLK], ps_rs)
                # AV: U[sq, d] = sum_sk expT[sk, sq]^T V[sk, d]
                for s4 in range(SQ4):
                    sqc = blk * SQ4 + s4
                    for nb in range(ND):
                        ps = ps_av.tile([P, DB], f32, name="ps_av")
                        for skc in range(KC):
                            nc.tensor.matmul(
                                ps,
                                exp_blk[:, skc, s4 * P:(s4 + 1) * P],
                                V_sb[:, skc, nb * DB:(nb + 1) * DB],
                                start=(skc == 0), stop=(skc == KC - 1),
                            )
                        o_sb = pout.tile([P, DB], f32, name="o_sb")
                        nc.vector.tensor_copy(o_sb, ps)
                        nc.sync.dma_start(
                            out=U.ap()[sqc * P:(sqc + 1) * P, nb * DB:(nb + 1) * DB],
                            in_=o_sb,
                        )
            nc.sync.dma_start(out=rs.ap().unsqueeze(0), in_=rs_stage[0:1, :])

    nc.finalize()
    _BUILD_CACHE[key] = nc
    return nc


def _numpy_fallback(x, Wk, bk, Wq, bq, Wv, bv, dims):
    k = x @ Wk + bk
    q = x @ Wq + bq
    v = x @ Wv + bv
    s = np.einsum("bqd,bkd->bqk", q, k) / np.sqrt(np.float32(q.shape[-1]))
    s = s - s.max(axis=dims, keepdims=True)
    e = np.exp(s)
    w = e / e.sum(axis=dims, keepdims=True)
    return np.einsum("bqk,bkd->bqd", w, v).astype(np.float32)


def kernel(x, Wk, bk, Wq, bq, Wv, bv, dims):
    x = np.asarray(x, np.float32)
    Wk = np.ascontiguousarray(np.asarray(Wk, np.float32))
    Wq = np.ascontiguousarray(np.asarray(Wq, np.float32))
    Wv = np.ascontiguousarray(np.asarray(Wv, np.float32))
    bk = np.ascontiguousarray(np.asarray(bk, np.float32))
    bq = np.ascontiguousarray(np.asarray(bq, np.float32))
    bv = np.ascontiguousarray(np.asarray(bv, np.float32))
    d = int(np.asarray(dims))
    if d != 2 or x.shape != (B, S, D):
        return _numpy_fallback(x, Wk, bk, Wq, bq, Wv, bv, d)

    from concourse.bass_utils import run_bass_kernel_spmd

    nc = _build(D, S // 2, S)

    if MM == "bfloat16":
        import ml_dtypes
        cast = lambda a: np.ascontiguousarray(a.astype(ml_dtypes.bfloat16))
    else:
        cast = np.ascontiguousarray

    Wks, Wqs, Wvs = cast(Wk), cast(Wq), cast(Wv)
    half = S // 2
    in_maps = []
    for c in range(N_CORES):
        b, h = c // 2, c % 2
        xT = x[b].T  # [D, S]
        in_maps.append({
            "xkvT": cast(xT[:, h * half:(h + 1) * half]),
            "xqT": cast(xT),
            "Wk": Wks, "Wq": Wqs, "Wv": Wvs, "bk": bk, "bq": bq,
        })

    res = run_bass_kernel_spmd(nc, in_maps, core_ids=list(range(N_CORES)),
                               **_RUN_KWARGS)
    global _LAST_RESULTS
    _LAST_RESULTS = res

    out = np.empty((B, S, D), np.float32)
    for b in range(B):
        r0, r1 = res.results[2 * b], res.results[2 * b + 1]
        num = r0["U"] + r1["U"]
        den = r0["rs"] + r1["rs"]
        out[b] = num / den[:, None] + bv
    return out
